# revision 21
# baseline (speedup 1.0000x reference)
"""Masked transformer encoder layer on 8 trn2 NeuronCores.

Sharding: pure data-parallel — batch B=8, one batch element per core, zero
collectives.  Each core runs the full layer on (N=1024, D=1024, H=16, F=4096).

Per-core pipeline (bf16 matmuls, fp32 accumulation / layernorm math):
  LN0 (token-major, bn_stats) -> h bf16 -> PE-transpose -> h^T (feature-major)
  q^T,k^T = Wqkv(q,k) @ h^T   (feature-major out)
  v       = h^T.T @ Wqkv(v)   (token-major out, +ones column for denominators)
  S^T     = k^T.T @ q^T   per head   (K=64, auto row-group packing)
  P^T     = exp(0.125*S^T + key_mask_bias)   (ACT, per-partition bias)
  out^T   = v_aug.T @ P^T  (row 64 = softmax denominator)
  attn^T  = out^T * bcast(1/denom)
  A       = attn^T.T @ Wproj^T (token-major) ; x1 = src + (1-mq)*w + mq*(A+bproj)
            (w = Wproj @ mean_j(v) + bproj handles fully-masked query rows)
  LN1 in-place (x1 -> x1n fp32) -> transpose -> x1n^T
  z^T     = W1 @ x1n^T ; gelu(+b1) ; y^T = W2 @ z^T
  out     = x1n + y^T.T + b2
"""

import numpy as np
import ml_dtypes

import concourse.bass as bass
import concourse.tile as tile
from concourse import bacc
from concourse import mybir
from concourse.bass_utils import run_bass_kernel_spmd

B, N, D, H, F = 8, 1024, 1024, 16, 4096
HD = D // H          # 64
P = 128
FC = D // P          # 8 feature chunks of D
TT = N // P          # 8 token tiles
GC = F // P          # 32 chunks of F
NEG = -1e30
EPS = 1e-5

f32 = mybir.dt.float32
bf16 = mybir.dt.bfloat16
AF = mybir.ActivationFunctionType
OP = mybir.AluOpType


def _layernorm_inplace_stats(nc, pools, x_ap):
    """Return (mean, rstd) APs ([128,1] each) for x_ap [128, 1024] fp32."""
    stats = pools["stats"].tile([P, 2, 6], f32)
    for sg in range(2):
        nc.vector.bn_stats(out=stats[:, sg, :], in_=x_ap[:, sg * 512:(sg + 1) * 512])
    mv = pools["mv"].tile([P, 2], f32)
    nc.vector.bn_aggr(out=mv[:], in_=stats[:])
    # rstd = 1/sqrt(var + eps)
    nc.scalar.activation(out=mv[:, 1:2], in_=mv[:, 1:2], func=AF.Sqrt,
                         bias=pools["eps"][:], scale=1.0)
    nc.vector.reciprocal(out=mv[:, 1:2], in_=mv[:, 1:2])
    return mv[:, 0:1], mv[:, 1:2]


def build_bass():
    nc = bacc.Bacc("TRN2")

    # ---------------- DRAM I/O ----------------
    src_h = nc.dram_tensor("src", [N, D], f32, kind="ExternalInput")
    kb_h = nc.dram_tensor("kbias", [TT, P], f32, kind="ExternalInput")
    mq_h = nc.dram_tensor("mq", [TT, P], f32, kind="ExternalInput")
    vecs_h = nc.dram_tensor("vecs", [6, D], f32, kind="ExternalInput")
    b1_h = nc.dram_tensor("b1r", [GC, P], f32, kind="ExternalInput")
    wqkv_h = nc.dram_tensor("wqkvT", [FC, P, 3 * D], bf16, kind="ExternalInput")
    wproj_h = nc.dram_tensor("wprojT", [FC, P, D], bf16, kind="ExternalInput")
    w1_h = nc.dram_tensor("w1T", [FC, P, F], bf16, kind="ExternalInput")
    w2_h = nc.dram_tensor("w2T", [GC, P, D], bf16, kind="ExternalInput")
    out_h = nc.dram_tensor("out", [N, D], f32, kind="ExternalOutput")

    with TileKernel(nc) as tk:
        tk.run(src_h, kb_h, mq_h, vecs_h, b1_h, wqkv_h, wproj_h, w1_h, w2_h, out_h)
    nc.compile()
    return nc


class TileKernel:
    def __init__(self, nc):
        self.nc = nc
        self.tc = tile.TileContext(nc)

    def __enter__(self):
        self.tc.__enter__()
        return self

    def __exit__(self, *a):
        return self.tc.__exit__(*a)

    def run(self, src_h, kb_h, mq_h, vecs_h, b1_h, wqkv_h, wproj_h, w1_h, w2_h, out_h):
        nc, tc = self.nc, self.tc
        from contextlib import ExitStack

        with ExitStack() as top:
            consts = top.enter_context(tc.tile_pool(name="consts", bufs=1))
            persist = top.enter_context(tc.tile_pool(name="persist", bufs=1))
            tmp_pool = top.enter_context(tc.tile_pool(name="tmp", bufs=2))
            stats_pool = top.enter_context(tc.tile_pool(name="stats", bufs=3))
            mv_pool = top.enter_context(tc.tile_pool(name="mv", bufs=4))
            
            # ---------- constants ----------
            ident = consts.tile([P, P], bf16)
            from concourse.masks import make_identity
            make_identity(nc, ident[:])
            ones_row = consts.tile([1, P], f32)
            nc.vector.memset(ones_row[:], 1.0)
            ones_col = consts.tile([P, 1], bf16)
            nc.vector.memset(ones_col[:], 1.0)
            ones_row_bf = consts.tile([1, P], bf16)
            nc.vector.memset(ones_row_bf[:], 1.0)
            eps_sb = consts.tile([P, 1], f32)
            nc.vector.memset(eps_sb[:], EPS)
            pools = {"stats": stats_pool, "mv": mv_pool, "eps": eps_sb}

            # DMA order tuned for startup: g0/beta0 broadcasts, then src
            # (LN0 gates everything), then the rest of the small constants.
            bcast = consts.tile([P, 6, D], f32)

            def _bcast_dma(v6):
                bc_src = bass.AP(tensor=vecs_h[0:1, :].tensor, offset=v6 * D,
                                 ap=[[0, P], [1, D]])
                nc.sync.dma_start(out=bcast[:, v6, :], in_=bc_src)

            for v6 in (0, 1):
                _bcast_dma(v6)

            src_sb = persist.tile([P, TT, D], f32)   # src -> srcw -> x1 -> x1n
            for tt in range(TT):
                nc.sync.dma_start(out=src_sb[:, tt, :],
                                  in_=src_h[tt * P:(tt + 1) * P, :])

            kb_sb = consts.tile([P, TT], f32)
            nc.sync.dma_start(out=kb_sb[:], in_=kb_h[:, :].rearrange("a p -> p a"))
            mq_sb = consts.tile([P, TT], f32)
            nc.sync.dma_start(out=mq_sb[:], in_=mq_h[:, :].rearrange("a p -> p a"))
            invmq_sb = consts.tile([P, TT], f32)
            nc.vector.tensor_scalar(out=invmq_sb[:], in0=mq_sb[:], scalar1=-1.0,
                                    scalar2=1.0, op0=OP.mult, op1=OP.add)
            b1_sb = consts.tile([P, GC], f32)
            nc.sync.dma_start(out=b1_sb[:], in_=b1_h[:, :].rearrange("g p -> p g"))
            for v6 in (2, 3, 4, 5):
                _bcast_dma(v6)
            g0b, beta0b = bcast[:, 0], bcast[:, 1]
            g1b, beta1b = bcast[:, 2], bcast[:, 3]
            bprojb, b2b = bcast[:, 4], bcast[:, 5]

            wb_sb = consts.tile([P, D], f32)       # (Wproj @ mean_j v + bproj) broadcast
            u_sb = consts.tile([P, FC], bf16)      # mean_j v, feature-major columns

            with ExitStack() as attn_scope:
                qkT = attn_scope.enter_context(tc.tile_pool(name="qkT", bufs=1))
                vp = attn_scope.enter_context(tc.tile_pool(name="vsb", bufs=1))

                qkT_sb = qkT.tile([P, 16, N], bf16)
                v_sb = vp.tile([P, TT, H, HD + 1], bf16)
                nc.vector.memset(v_sb[:, :, :, HD:HD + 1], 1.0)

                # ================= LN0 + transpose + QKV =================
                with ExitStack() as qkv_scope:
                    htp = qkv_scope.enter_context(tc.tile_pool(name="hT", bufs=1))
                    hbp = qkv_scope.enter_context(tc.tile_pool(name="hbf", bufs=2))
                    trps = qkv_scope.enter_context(
                        tc.tile_pool(name="trps", bufs=2, space="PSUM"))
                    qkps = qkv_scope.enter_context(
                        tc.tile_pool(name="qkps", bufs=3, space="PSUM"))

                    hT_sb = htp.tile([P, FC, N], bf16)

                    for tt in range(TT):
                        x = src_sb[:, tt, :]
                        mean, rstd = _layernorm_inplace_stats(nc, pools, x)
                        ht = tmp_pool.tile([P, D], f32, tag="lnt")
                        nc.vector.tensor_scalar(out=ht[:], in0=x, scalar1=mean,
                                                scalar2=rstd, op0=OP.subtract, op1=OP.mult)
                        nc.gpsimd.tensor_tensor(ht[:], ht[:], g0b, OP.mult)
                        hbf = hbp.tile([P, D], bf16)
                        nc.vector.tensor_tensor(hbf[:], ht[:], beta0b, OP.add)
                        for fb in range(FC):
                            ps = trps.tile([P, P], bf16)
                            nc.tensor.transpose(ps[:], hbf[:, fb * P:(fb + 1) * P], ident[:])
                            nc.scalar.copy(hT_sb[:, fb, tt * P:(tt + 1) * P], ps[:])

                    # q^T, k^T (feature-major)
                    with tc.tile_pool(name="wqk", bufs=1) as wqkp:
                        wqk_sb = wqkp.tile([P, FC, 2 * D], bf16)
                        for fc in range(FC):
                            nc.sync.dma_start(out=wqk_sb[:, fc, :],
                                              in_=wqkv_h[fc, :, 0:2 * D])
                        for oc in range(16):
                            ps = qkps.tile([P, 1024], f32)
                            for ib in range(2):
                                for fc in range(FC):
                                    nc.tensor.matmul(
                                        ps[:, ib * 512:(ib + 1) * 512],
                                        wqk_sb[:, fc, oc * P:(oc + 1) * P],
                                        hT_sb[:, fc, ib * 512:(ib + 1) * 512],
                                        start=(fc == 0), stop=(fc == FC - 1))
                            nc.vector.tensor_copy(qkT_sb[:, oc, :], ps[:])

                    # v (token-major) into per-head lhsT layout
                    with tc.tile_pool(name="wv", bufs=1) as wvp:
                        wv_sb = wvp.tile([P, FC, D], bf16)
                        nc.sync.dma_start(out=wv_sb[:],
                                          in_=wqkv_h[:, :, 2 * D:3 * D].rearrange("f p o -> p f o"))
                        for tt in range(TT):
                            ps = qkps.tile([P, 1024], f32)
                            for vb in range(2):
                                for fc in range(FC):
                                    nc.tensor.matmul(
                                        ps[:, vb * 512:(vb + 1) * 512],
                                        hT_sb[:, fc, tt * P:(tt + 1) * P],
                                        wv_sb[:, fc, vb * 512:(vb + 1) * 512],
                                        start=(fc == 0), stop=(fc == FC - 1))
                            nc.vector.tensor_copy(
                                v_sb[:, tt, :, 0:HD],
                                ps[:].rearrange("p (h c) -> p h c", h=H))

                atp = attn_scope.enter_context(tc.tile_pool(name="attnT", bufs=1))
                wpp = attn_scope.enter_context(tc.tile_pool(name="wproj", bufs=1))
                attnT_sb = atp.tile([P, FC, N], bf16)
                wproj_sb = wpp.tile([P, FC, D], bf16)
                nc.sync.dma_start(out=wproj_sb[:],
                                  in_=wproj_h[:, :, :].rearrange("f p o -> p f o"))

                # ============ u = mean_j v ; w = Wproj @ u + bproj ============
                with tc.tile_pool(name="uwps", bufs=2, space="PSUM") as uwps, \
                        tc.tile_pool(name="wrowp", bufs=1) as wrowp:
                    wrow = wrowp.tile([1, D], f32)
                    for fc in range(FC):
                        ps = uwps.tile([P, 512], f32, tag="ups")
                        for hh in range(2):
                            for jc in range(TT):
                                nc.tensor.matmul(ps[hh * HD:(hh + 1) * HD, 0:1],
                                                 v_sb[:, jc, 2 * fc + hh, 0:HD],
                                                 ones_col[:],
                                                 start=(jc == 0), stop=(jc == TT - 1))
                        nc.vector.tensor_scalar(out=u_sb[:, fc:fc + 1], in0=ps[:, 0:1],
                                                scalar1=1.0 / N, scalar2=None, op0=OP.mult)
                    for ob in range(2):
                        ps = uwps.tile([P, 512], f32, tag="wps")
                        for fc in range(FC):
                            nc.tensor.matmul(ps[0:1, :], u_sb[:, fc:fc + 1],
                                             wproj_sb[:, fc, ob * 512:(ob + 1) * 512],
                                             start=(fc == 0), stop=(fc == FC - 1))
                        nc.vector.tensor_tensor(wrow[:, ob * 512:(ob + 1) * 512], ps[0:1, :],
                                                bprojb[0:1, ob * 512:(ob + 1) * 512], OP.add)
                    for ob in range(2):
                        ps = uwps.tile([P, 512], f32, tag="wbps")
                        nc.tensor.matmul(ps[:], ones_row[:],
                                         wrow[:, ob * 512:(ob + 1) * 512],
                                         start=True, stop=True)
                        nc.vector.tensor_copy(wb_sb[:, ob * 512:(ob + 1) * 512], ps[:])

                # srcw = src + (1-mq)*wb + mq*bprojb   (in place)
                for tt in range(TT):
                    t = tmp_pool.tile([P, D], f32, tag="srcw")
                    nc.vector.tensor_scalar(out=t[:], in0=wb_sb[:],
                                            scalar1=invmq_sb[:, tt:tt + 1],
                                            scalar2=None, op0=OP.mult)
                    nc.vector.tensor_tensor(src_sb[:, tt, :], src_sb[:, tt, :], t[:], OP.add)
                    t2 = tmp_pool.tile([P, D], f32, tag="srcw")
                    nc.gpsimd.tensor_scalar(out=t2[:], in0=bprojb[:],
                                            scalar1=mq_sb[:, tt:tt + 1],
                                            scalar2=None, op0=OP.mult)
                    nc.gpsimd.tensor_tensor(src_sb[:, tt, :], src_sb[:, tt, :], t2[:], OP.add)

                # ================= attention + proj =================
                with ExitStack() as att:
                    ptp = att.enter_context(tc.tile_pool(name="pt", bufs=12))
                    rdp = att.enter_context(tc.tile_pool(name="rd", bufs=3))
                    dnp = att.enter_context(tc.tile_pool(name="dn", bufs=3))
                    sps = att.enter_context(tc.tile_pool(name="sps", bufs=2, space="PSUM"))
                    avps = att.enter_context(tc.tile_pool(name="avps", bufs=3, space="PSUM"))
                    bcps2 = att.enter_context(tc.tile_pool(name="bcps2", bufs=1, space="PSUM"))

                    for h in range(H):
                        hp = (h % 2) * HD
                        fc_h = h // 2
                        # S^T for both i-blocks into one 2-bank psum; single exp
                        pts = []
                        for jc in range(TT):
                            ps_s = sps.tile([P, 1024], f32)
                            for ib in range(2):
                                nc.tensor.matmul(
                                    ps_s[:, ib * 512:(ib + 1) * 512],
                                    qkT_sb[hp:hp + HD, 8 + fc_h, jc * P:(jc + 1) * P],
                                    qkT_sb[hp:hp + HD, fc_h, ib * 512:(ib + 1) * 512],
                                    start=True, stop=True)
                            pt = ptp.tile([P, 1024], bf16)
                            nc.scalar.activation(out=pt[:], in_=ps_s[:], func=AF.Exp,
                                                 bias=kb_sb[:, jc:jc + 1], scale=0.125)
                            pts.append(pt)
                        for ib in range(2):
                            isl = slice(ib * 512, (ib + 1) * 512)
                            ps_av = avps.tile([P, 512], f32)
                            for jc in range(TT):
                                nc.tensor.matmul(ps_av[0:HD + 1, :],
                                                 v_sb[:, jc, h, :], pts[jc][:, isl],
                                                 start=(jc == 0), stop=(jc == TT - 1))
                            dn = dnp.tile([1, 512], bf16)
                            nc.vector.tensor_copy(dn[:], ps_av[HD:HD + 1, :])
                            ps_b = bcps2.tile([P, 512], f32)
                            nc.tensor.matmul(ps_b[:], ones_row_bf[:], dn[:],
                                             start=True, stop=True)
                            rd = rdp.tile([P, 512], f32)
                            nc.vector.reciprocal(rd[:], ps_b[:])
                            nc.vector.tensor_tensor(
                                attnT_sb[hp:hp + HD, fc_h, isl],
                                ps_av[0:HD, :], rd[0:HD, :], OP.mult)



                # proj + x1 (into src_sb); own psum scope after attention frees banks
                with tc.tile_pool(name="pps", bufs=3, space="PSUM") as pps:
                    for tt in range(TT):
                        for ob in range(2):
                            osl = slice(ob * 512, (ob + 1) * 512)
                            ps_p = pps.tile([P, 512], f32)
                            for fc in range(FC):
                                nc.tensor.matmul(ps_p[:],
                                                 attnT_sb[:, fc, tt * P:(tt + 1) * P],
                                                 wproj_sb[:, fc, osl],
                                                 start=(fc == 0), stop=(fc == FC - 1))
                            t = tmp_pool.tile([P, 512], f32, tag="x1t")
                            nc.vector.tensor_scalar(out=t[:], in0=ps_p[:],
                                                    scalar1=mq_sb[:, tt:tt + 1],
                                                    scalar2=None, op0=OP.mult)
                            nc.vector.tensor_tensor(src_sb[:, tt, osl],
                                                    src_sb[:, tt, osl], t[:], OP.add)

            # ================= LN1 (in place) + transpose =================
            with ExitStack() as ffn1:
                ztp = ffn1.enter_context(tc.tile_pool(name="zT", bufs=1))
                zT_sb = ztp.tile([P, GC, N], bf16)
                f1 = ffn1.enter_context(ExitStack())
                xtp = f1.enter_context(tc.tile_pool(name="x1nT", bufs=1))
                xbp = f1.enter_context(tc.tile_pool(name="x1nbf", bufs=2))
                w1p = f1.enter_context(tc.tile_pool(name="w1p", bufs=3))
                trps2 = f1.enter_context(tc.tile_pool(name="trps2", bufs=3, space="PSUM"))
                zps = f1.enter_context(tc.tile_pool(name="zps", bufs=2, space="PSUM"))

                x1nT_sb = xtp.tile([P, FC, N], bf16)

                for tt in range(TT):
                    x = src_sb[:, tt, :]
                    mean, rstd = _layernorm_inplace_stats(nc, pools, x)
                    nc.vector.tensor_scalar(out=x, in0=x, scalar1=mean,
                                            scalar2=rstd, op0=OP.subtract, op1=OP.mult)
                    nc.gpsimd.tensor_tensor(x, x, g1b, OP.mult)
                    nc.vector.tensor_tensor(x, x, beta1b, OP.add)
                    xbf = xbp.tile([P, D], bf16)
                    nc.gpsimd.tensor_copy(xbf[:], x)
                    for fb in range(FC):
                        ps = trps2.tile([P, P], bf16)
                        nc.tensor.transpose(ps[:], xbf[:, fb * P:(fb + 1) * P], ident[:])
                        nc.scalar.copy(x1nT_sb[:, fb, tt * P:(tt + 1) * P], ps[:])

                # ---------------- FFN linear1 + gelu ----------------
                for gc in range(GC):
                    w1t = w1p.tile([P, FC, P], bf16)
                    nc.sync.dma_start(out=w1t[:],
                                      in_=w1_h[:, :, gc * P:(gc + 1) * P].rearrange("f p c -> p f c"))
                    ps = zps.tile([P, 1024], f32)
                    for ib in range(2):
                        for fc in range(FC):
                            nc.tensor.matmul(ps[:, ib * 512:(ib + 1) * 512],
                                             w1t[:, fc, :],
                                             x1nT_sb[:, fc, ib * 512:(ib + 1) * 512],
                                             start=(fc == 0), stop=(fc == FC - 1))
                    nc.scalar.activation(out=zT_sb[:, gc, :], in_=ps[:], func=AF.Gelu,
                                         bias=b1_sb[:, gc:gc + 1], scale=1.0)

                # -------- FFN linear2, token-major y, fused residual+out --------
                f1.close()
                with ExitStack() as ffn2:
                    w2p = ffn2.enter_context(tc.tile_pool(name="w2p", bufs=2))
                    yout = ffn2.enter_context(tc.tile_pool(name="yout", bufs=4))
                    yps = ffn2.enter_context(tc.tile_pool(name="yps", bufs=3, space="PSUM"))

                    QW = 256
                    for ob in range(4):
                        osl = slice(ob * QW, (ob + 1) * QW)
                        w2q = w2p.tile([P, GC, QW], bf16)
                        nc.sync.dma_start(out=w2q[:],
                                          in_=w2_h[:, :, osl].rearrange("g p c -> p g c"))
                        for tt in range(TT):
                            ps = yps.tile([P, QW], f32)
                            for gc in range(GC):
                                nc.tensor.matmul(ps[:],
                                                 zT_sb[:, gc, tt * P:(tt + 1) * P],
                                                 w2q[:, gc, :],
                                                 start=(gc == 0), stop=(gc == GC - 1))
                            t = yout.tile([P, QW], f32)
                            nc.vector.tensor_tensor(t[:], ps[:], b2b[:, osl], OP.add)
                            nc.vector.tensor_tensor(t[:], t[:], src_sb[:, tt, osl], OP.add)
                            nc.sync.dma_start(out=out_h[tt * P:(tt + 1) * P, osl], in_=t[:])


_NC_CACHE = {}


def _get_nc():
    if "nc" not in _NC_CACHE:
        _NC_CACHE["nc"] = build_bass()
    return _NC_CACHE["nc"]


def prep_in_maps(inputs):
    src = np.asarray(inputs["src"], dtype=np.float32)          # [B, N, D]
    mask = np.asarray(inputs["mask"])                          # [B, N] bool
    Wqkv = np.asarray(inputs["Wqkv"], dtype=np.float32)
    Wproj = np.asarray(inputs["Wproj"], dtype=np.float32)
    bproj = np.asarray(inputs["bproj"], dtype=np.float32)
    W1 = np.asarray(inputs["W1"], dtype=np.float32)
    b1 = np.asarray(inputs["b1"], dtype=np.float32)
    W2 = np.asarray(inputs["W2"], dtype=np.float32)
    b2 = np.asarray(inputs["b2"], dtype=np.float32)
    g0 = np.asarray(inputs["g0"], dtype=np.float32)
    beta0 = np.asarray(inputs["beta0"], dtype=np.float32)
    g1 = np.asarray(inputs["g1"], dtype=np.float32)
    beta1 = np.asarray(inputs["beta1"], dtype=np.float32)

    bf = ml_dtypes.bfloat16
    wqkvT = np.ascontiguousarray(Wqkv.T).reshape(FC, P, 3 * D).astype(bf)
    wprojT = np.ascontiguousarray(Wproj.T).reshape(FC, P, D).astype(bf)
    w1T = np.ascontiguousarray(W1.T).reshape(FC, P, F).astype(bf)
    w2T = np.ascontiguousarray(W2.T).reshape(GC, P, D).astype(bf)
    vecs = np.ascontiguousarray(np.stack([g0, beta0, g1, beta1, bproj, b2]))
    b1r = np.ascontiguousarray(b1.reshape(GC, P))
    kbias = np.where(mask, 0.0, NEG).astype(np.float32).reshape(B, TT, P)
    mqf = mask.astype(np.float32).reshape(B, TT, P)

    in_maps = []
    for b in range(B):
        in_maps.append({
            "src": np.ascontiguousarray(src[b]),
            "kbias": np.ascontiguousarray(kbias[b]),
            "mq": np.ascontiguousarray(mqf[b]),
            "vecs": vecs,
            "b1r": b1r,
            "wqkvT": wqkvT,
            "wprojT": wprojT,
            "w1T": w1T,
            "w2T": w2T,
        })
    return in_maps


def kernel(**inputs):
    in_maps = prep_in_maps(inputs)
    nc = _get_nc()
    res = run_bass_kernel_spmd(nc, in_maps, core_ids=list(range(B)))
    return np.stack([r["out"] for r in res.results]).astype(np.float32)
